# revision 13
# baseline (speedup 1.0000x reference)
"""EventVolumeSurface trilinear voxel-grid kernel for Trainium2 (Bass/Tile).

Strategy (data-parallel over batch, 1 batch -> 1 NeuronCore):
  Host: shard events by batch id, bucket by (time-segment s in [0,9),
  y-stripe q in [0,8) of 64 rows; events straddling a y-stripe boundary are
  duplicated - the trilinear hat masks the out-of-stripe tap), then SORT each
  bucket's events by x.  Cut into 128-event tiles; each tile gets a
  compile-time x-window [base_t, base_t+w_t) covering all 8 cores' taps
  (w_t ~ 16-40 cols).  Pack slot-major [128, T] arrays of y, x-base-relative,
  t, polarity.

  Device, per tile of 128 events (ops batched over groups of <=8 tiles):
    dy   = iota_y - y            (Pool,  [128, G*64])
    ady  = |dy|                  (ACT Abs)
    nhy  = min(ady,1) - 1        (DVE fused tensor_scalar, fp16) = -hat_y
    dx   = iota_w - xloc         (Pool/DVE)
    adx  = |dx|                  (ACT Abs)
    mxc  = min(adx,1)            (DVE, fp16)
    r0   = g0*mxc - g0           (DVE fused 2-scalar TS, fp16) = -g0*hat_x
    r1   = g1*mxc - g1           where g0 = p*(1-frac), g1 = p*frac
    psum[64, 1280] += nhy^T @ [r0 | r1]  (PE, N=w_t per bin half)
  so psum accumulates +g*hat_y*hat_x for the two adjacent bin planes.  PSUM
  is drained per (s,q) into an SBUF-resident [10,480,640] grid (pre-zeroed)
  which is DMA'd out plane-by-plane as planes finalize.
"""

import os
import sys

import numpy as np

sys.path.insert(0, "/opt/trn_rl_repo")

import concourse.bass as bass
import concourse.bacc as bacc
import concourse.mybir as mybir
import concourse.tile as tile
from concourse.bass_utils import run_bass_kernel_spmd

H, W, BINS = 480, 640, 10
NSEG = BINS - 1          # 9 time segments (events with t*=9 fold into seg 8)
P = 128
QS = 64                  # y-stripe height
NQ8 = 8                  # ceil(480/64) = 7.5 -> 8 stripes (last half-used)
NKEY = NSEG * NQ8        # 72 buckets
NQ4 = 4                  # V-grid column blocks of 128 rows
N_CORES = 8
GROUP = 8                # tiles per batched op
XCAP = 1280              # max f32 columns per batched x-op slab (SBUF budget)

F32 = mybir.dt.float32
F16 = mybir.dt.float16

# fraction (numerator/16) of per-tile r-ops issued on Pool engine (rest DVE)
R_POOL_NUM = int(os.environ.get("EVS_R_POOL_NUM", "7"))
# fraction (numerator/4) of plane drain-copies issued on DVE (rest ACT)
DR_DVE_NUM = int(os.environ.get("EVS_DR_DVE_NUM", "2"))

_prog_cache: dict = {}


def _host_prep(ev):
    """Bucket one batch's events by (s, q64); returns counts and raw data."""
    if ev.shape[0] == 0:
        ev = np.array([[0.0, 0.0, 0.25, 0.0, 0.0],
                       [0.0, 0.0, 0.75, 0.0, 0.0]], np.float32)
    x = ev[:, 0].astype(np.float32)
    y = ev[:, 1].astype(np.float32)
    t = ev[:, 2].astype(np.float32)
    p = ev[:, 3].astype(np.float32)
    t0 = t[0]
    tN = t[-1]
    denom = np.float32(tN - t0)
    if denom > 0:
        a = np.float32(np.float32(NSEG) / denom)
    else:
        a = np.float32(0.0)
    b = np.float32(-t0 * a)
    tp = (t * a + b).astype(np.float32)
    s = np.clip(np.floor(tp).astype(np.int32), 0, NSEG - 1)

    iy = np.floor(y).astype(np.int32)
    qf = iy >> 6
    qc = (iy + 1) >> 6
    n = len(x)
    idx0 = np.arange(n, dtype=np.int64)
    ys = qf != qc
    inst_idx = np.concatenate([idx0, idx0[ys]])
    inst_q = np.concatenate([qf, qc[ys]])
    key = s[inst_idx] * NQ8 + inst_q
    counts = np.bincount(key, minlength=NKEY)
    return counts, (x, y, t, p, a, b, inst_idx, key)


def _assign_slots(pack, tiles_per_key):
    """Sort instances by (key, x) and assign (partition, tile-col) slots."""
    x, y, t, p, a, b, inst_idx, key = pack
    col0 = np.zeros(NKEY + 1, np.int64)
    col0[1:] = np.cumsum(tiles_per_key)
    order = np.lexsort((x[inst_idx], key))
    skey = key[order]
    sidx = inst_idx[order]
    group_start = np.searchsorted(skey, np.arange(NKEY + 1))
    nk = np.diff(group_start)                      # this core's bucket counts
    rank = np.arange(len(skey)) - group_start[skey]
    # proportional-rank cut: tile j gets ranks [ceil(j*n/T), ceil((j+1)*n/T))
    # so each core's tiles cover aligned x-quantiles (narrower shared window)
    tk = tiles_per_key[skey]
    nks = np.maximum(nk[skey], 1)
    tile_in_key = (rank * tk) // nks
    # position within tile
    j0 = -(-(tile_in_key * nks) // tk)             # ceil(j*n/T)
    part = (rank - j0).astype(np.int64)
    col = (col0[skey] + tile_in_key).astype(np.int64)
    assert part.max(initial=0) < P
    return part, col, sidx, (x, y, t, p, a, b)


def _pack_core(slots, base, T_tot, key_of_col):
    part, col, sidx, (x, y, t, p, a, b) = slots
    xv = x[sidx]
    yv = y[sidx]
    tv = t[sidx]
    pv = p[sidx]

    qlo_col = ((key_of_col % NQ8) * QS).astype(np.float32)
    YLOC = np.zeros((P, T_tot), np.float32)
    YLOC[part, col] = yv - qlo_col[col]
    XLOC = np.zeros((P, T_tot), np.float32)
    XLOC[part, col] = xv - base[col].astype(np.float32)
    TP = np.zeros((P, 2 * T_tot + 2), np.float32)
    TP[part, col] = tv
    TP[part, T_tot + col] = pv
    TP[:, 2 * T_tot] = a
    TP[:, 2 * T_tot + 1] = b

    # K=3 fp16 lhsT slabs, rows [1, hi, lo], laid out [3, T*128] tile-major
    def split3(v):
        out = np.zeros((3, T_tot * P), np.float16)
        out[0] = 1.0
        hi = v.astype(np.float16)
        lo = (v - hi.astype(np.float32)).astype(np.float16)
        out[1].reshape(T_tot, P)[:] = hi.T
        out[2].reshape(T_tot, P)[:] = lo.T
        return out

    return {"ev_tp": TP, "yl3": split3(YLOC), "xl3": split3(XLOC)}


def _windows(slots, T_tot):
    """Shared per-tile x-window [base, base+width) covering all cores."""
    minx = np.full(T_tot, W, np.int64)
    maxe = np.zeros(T_tot, np.int64)
    for part, col, sidx, (x, y, t, p, a, b) in slots:
        fx = np.floor(x[sidx]).astype(np.int64)
        np.minimum.at(minx, col, fx)
        np.maximum.at(maxe, col, fx + 2)
    base = np.maximum(np.minimum(minx, W - 4), 0)
    end = np.minimum(np.maximum(maxe, base + 4), W)
    width = np.minimum((end - base + 1) // 2 * 2, W - base)
    return base, width


def _build_program(tiles_per_key, base, width, T_tot, WXM):
    nc = bacc.Bacc("TRN2", debug=False)
    tp_d = nc.dram_tensor("ev_tp", [P, 2 * T_tot + 2], F32,
                          kind="ExternalInput")
    yl3_d = nc.dram_tensor("yl3", [3, T_tot * P], F16, kind="ExternalInput")
    xl3_d = nc.dram_tensor("xl3", [3, T_tot * P], F16, kind="ExternalInput")
    out_d = nc.dram_tensor("out", [BINS, H, W], F32, kind="ExternalOutput")
    MAXT = int(tiles_per_key.max())

    col0 = np.zeros(NKEY + 1, np.int64)
    col0[1:] = np.cumsum(tiles_per_key)
    seg_c0 = [int(col0[s * NQ8]) for s in range(NSEG)]
    seg_c1 = [int(col0[(s + 1) * NQ8]) for s in range(NSEG)]

    Alu = mybir.AluOpType
    Act = mybir.ActivationFunctionType

    with tile.TileContext(nc) as tc:
        with (
            tc.tile_pool(name="persist", bufs=1) as persist,
            tc.tile_pool(name="psum", bufs=2, space="PSUM") as psump,
        ):
            # K=3 matmul rhs constants: row0 = local iota (fp16 exact to
            # 2048), rows 1-2 = -1
            ioi = persist.tile([P, W], mybir.dt.int32, tag="ioi")
            nc.gpsimd.iota(ioi[:], pattern=[[1, W]], base=0,
                           channel_multiplier=0)
            rio = persist.tile([3, W], F16, tag="rio")
            nc.vector.memset(rio[:], -1.0)
            nc.vector.tensor_copy(rio[0:1, :], ioi[0:1, :])

            # prologue: frac = a*t + b - s ; g1 = p*frac ; g0 = p - g1
            g0p = persist.tile([P, T_tot], F32, tag="g0p")
            g1p = persist.tile([P, T_tot], F32, tag="g1p")
            with tc.tile_pool(name="prolog", bufs=1) as prolog:
                tpt = prolog.tile([P, 2 * T_tot + 2], F32, tag="tpt")
                tt = tpt[:, 0:T_tot]
                pt = tpt[:, T_tot:2 * T_tot]
                ab = tpt[:, 2 * T_tot:2 * T_tot + 2]
                nc.sync.dma_start(out=tpt[:], in_=tp_d[:])
                tc.strict_bb_all_engine_barrier()
                frac = prolog.tile([P, T_tot], F32, tag="frac")
                nc.vector.tensor_scalar(frac[:], tt, ab[:, 0:1], ab[:, 1:2],
                                        op0=Alu.mult, op1=Alu.add)
                for s in range(NSEG):
                    c0, c1 = seg_c0[s], seg_c1[s]
                    if c1 > c0 and s > 0:
                        nc.vector.tensor_scalar(frac[:, c0:c1], frac[:, c0:c1],
                                                float(s), None,
                                                op0=Alu.subtract)
                nc.vector.tensor_tensor(g1p[:], pt, frac[:], op=Alu.mult)
                nc.vector.tensor_tensor(g0p[:], pt, g1p[:], op=Alu.subtract)

            # output grid: every (plane, q8) region is first written by a
            # copy-drain, so no zero-init is needed
            z16 = persist.tile([P, 512], F16, tag="z16")
            nc.vector.memset(z16[:], 0.0)

            tc.strict_bb_all_engine_barrier()

            repeat = int(os.environ.get("EVS_REPEAT", "1"))
            r_rr = 0
            dr_rr = 0
            XBANKS = (0, 512, W)
            with (
                tc.tile_pool(name="ylst", bufs=2) as ylstp,
                tc.tile_pool(name="xlst", bufs=2) as xlstp,
                tc.tile_pool(name="dyps", bufs=2, space="PSUM") as dypsp,
                tc.tile_pool(name="dxps", bufs=2, space="PSUM") as dxpsp,
                tc.tile_pool(name="adyp", bufs=3) as adyp,
                tc.tile_pool(name="nhyp", bufs=3) as nhyp,
                tc.tile_pool(name="adxp", bufs=2) as adxp,
                tc.tile_pool(name="mxcp", bufs=2) as mxcp,
                tc.tile_pool(name="rp", bufs=8) as rp,
                tc.tile_pool(name="stgp", bufs=2) as stgp,
            ):
             for _rep in range(repeat):
              for q8 in range(NQ8):
                pr0 = (q8 & 1) * QS
                rows = min(QS, H - q8 * QS)

                # plan all mm pieces for this q8 stripe: piece key =
                # (s, half, tile, p0, p1); per output plane+bank find the
                # last writer (gets stop=True) and which banks are untouched
                def pieces_of(s, half):
                    k = s * NQ8 + q8
                    out = []
                    for t_i in range(int(tiles_per_key[k])):
                        c = int(col0[k]) + t_i
                        w = int(width[c])
                        bs = int(base[c])
                        cur = bs
                        while cur < bs + w:
                            nxt_b = min(bs + w,
                                        next(b for b in XBANKS[1:] if b > cur))
                            out.append((t_i, cur, nxt_b))
                            cur = nxt_b
                    return out

                last_set = set()
                empty_banks = {}   # plane -> set of untouched bank indices
                for plane in range(BINS):
                    empty_banks[plane] = set()
                    for bk in range(2):
                        b0, b1 = XBANKS[bk], XBANKS[bk + 1]
                        writers = []
                        if plane >= 1:
                            writers += [(plane - 1, 1, t, p0, p1)
                                        for (t, p0, p1) in pieces_of(plane - 1, 1)
                                        if b0 <= p0 < b1]
                        if plane <= NSEG - 1:
                            writers += [(plane, 0, t, p0, p1)
                                        for (t, p0, p1) in pieces_of(plane, 0)
                                        if b0 <= p0 < b1]
                        if writers:
                            last_set.add(writers[-1])
                        else:
                            empty_banks[plane].add(bk)

                def new_plane_tile(plane):
                    ps = psump.tile([P, W], F32, tag="pp")
                    for bk in range(2):
                        b0, b1 = XBANKS[bk], XBANKS[bk + 1]
                        nc.tensor.matmul(ps[pr0:pr0 + QS, b0:b1],
                                         lhsT=z16[:, 0:QS],
                                         rhs=z16[:, 0:b1 - b0],
                                         start=True,
                                         stop=(bk in empty_banks[plane]))
                    return ps

                ptile = {0: new_plane_tile(0)}
                for s in range(NSEG):
                    ptile[s + 1] = new_plane_tile(s + 1)
                    k = s * NQ8 + q8
                    ntile = int(tiles_per_key[k])
                    cbase = int(col0[k])

                    # stream this bucket's lhsT slabs from DRAM
                    ylst = ylstp.tile([3, MAXT * P], F16, tag="ylst")
                    xlst = xlstp.tile([3, MAXT * P], F16, tag="xlst")
                    nc.sync.dma_start(
                        out=ylst[:, 0:ntile * P],
                        in_=yl3_d[:, cbase * P:(cbase + ntile) * P])
                    nc.sync.dma_start(
                        out=xlst[:, 0:ntile * P],
                        in_=xl3_d[:, cbase * P:(cbase + ntile) * P])

                    g0 = 0
                    while g0 < ntile:
                        gn = 1
                        wg = int(width[cbase + g0])
                        while (g0 + gn < ntile and gn < GROUP):
                            w2 = max(wg, int(width[cbase + g0 + gn]))
                            if (gn + 1) * w2 > 512:
                                break
                            wg = w2
                            gn += 1
                        gstart = g0
                        c0 = cbase + gstart
                        g0 += gn

                        # y side: dy (PE), ady16 (ACT from psum), nhy16 (DVE)
                        dyG = dypsp.tile([P, GROUP * QS], F32, tag="dyG")
                        for j in range(gn):
                            t_i = gstart + j
                            nc.tensor.matmul(
                                dyG[:, j * QS:(j + 1) * QS],
                                lhsT=ylst[:, t_i * P:(t_i + 1) * P],
                                rhs=rio[:, 0:QS], start=True, stop=True)
                        adyS = adyp.tile([P, GROUP * QS], F16, tag="adyS")
                        nc.scalar.activation(adyS[:, 0:gn * QS],
                                             dyG[:, 0:gn * QS], Act.Abs)
                        nhyS = nhyp.tile([P, GROUP * QS], F16, tag="nhyS")
                        nc.vector.tensor_scalar(nhyS[:, 0:gn * QS],
                                                adyS[:, 0:gn * QS], 1.0, 1.0,
                                                op0=Alu.min, op1=Alu.subtract)

                        # x side: dx (PE), adx16 (ACT from psum), mxc16 (DVE)
                        dxG = dxpsp.tile([P, 512], F32, tag="dxG")
                        for j in range(gn):
                            t_i = gstart + j
                            w = int(width[cbase + t_i])
                            nc.tensor.matmul(
                                dxG[:, j * wg:j * wg + w],
                                lhsT=xlst[:, t_i * P:(t_i + 1) * P],
                                rhs=rio[:, 0:w], start=True, stop=True)
                        adxS = adxp.tile([P, 512], F16, tag="adxS")
                        nc.scalar.activation(adxS[:, 0:gn * wg],
                                             dxG[:, 0:gn * wg], Act.Abs)
                        mxcS = mxcp.tile([P, 512], F16, tag="mxcS")
                        nc.vector.tensor_scalar(mxcS[:, 0:gn * wg],
                                                adxS[:, 0:gn * wg], 1.0, None,
                                                op0=Alu.min)

                        for j in range(gn):
                            t_i = gstart + j
                            c = cbase + t_i
                            w = int(width[c])
                            bs = int(base[c])
                            mx_j = mxcS[:, j * wg:j * wg + w]
                            for half, gcol in ((0, g0p), (1, g1p)):
                                ps = ptile[s + half]
                                rr_t = rp.tile([P, 640], F16, tag="rr")
                                rrw = rr_t[:, 0:w]
                                if r_rr < R_POOL_NUM:
                                    nc.gpsimd.tensor_scalar(
                                        rrw, mx_j, gcol[:, c:c + 1],
                                        gcol[:, c:c + 1],
                                        op0=Alu.mult, op1=Alu.subtract)
                                else:
                                    nc.vector.tensor_scalar(
                                        rrw, mx_j, gcol[:, c:c + 1],
                                        gcol[:, c:c + 1],
                                        op0=Alu.mult, op1=Alu.subtract)
                                r_rr = (r_rr + 1) & 15
                                cur = bs
                                while cur < bs + w:
                                    pe = min(bs + w,
                                             next(b for b in XBANKS[1:]
                                                  if b > cur))
                                    is_last = (s, half, t_i, cur, pe) \
                                        in last_set
                                    nc.tensor.matmul(
                                        ps[pr0:pr0 + QS, cur:pe],
                                        lhsT=nhyS[:, j * QS:(j + 1) * QS],
                                        rhs=rr_t[:, cur - bs:pe - bs],
                                        start=False, stop=is_last)
                                    cur = pe

                    # plane s complete: copy psum -> staging, DMA out
                    stg = stgp.tile([P, W], F32, tag="stg")
                    if dr_rr < DR_DVE_NUM:
                        nc.vector.tensor_copy(stg[pr0:pr0 + QS, :],
                                              ptile[s][pr0:pr0 + QS, :])
                    else:
                        nc.scalar.copy(stg[pr0:pr0 + QS, :],
                                       ptile[s][pr0:pr0 + QS, :])
                    dr_rr = (dr_rr + 1) & 3
                    if _rep == repeat - 1 and rows > 0:
                        nc.sync.dma_start(
                            out=out_d[s, q8 * QS:q8 * QS + rows, :],
                            in_=stg[pr0:pr0 + rows, :])
                    del ptile[s]

                stg = stgp.tile([P, W], F32, tag="stg")
                nc.scalar.copy(stg[pr0:pr0 + QS, :],
                               ptile[NSEG][pr0:pr0 + QS, :])
                if _rep == repeat - 1 and rows > 0:
                    nc.sync.dma_start(
                        out=out_d[NSEG, q8 * QS:q8 * QS + rows, :],
                        in_=stg[pr0:pr0 + rows, :])
                del ptile[NSEG]
    nc.finalize()
    return nc


def kernel(events, lengths):
    events = np.ascontiguousarray(events, dtype=np.float32)
    lengths = np.asarray(lengths)
    B = int(lengths.shape[0])
    offs = np.zeros(B + 1, np.int64)
    offs[1:] = np.cumsum(lengths)

    packs = []
    counts = np.zeros((B, NKEY), np.int64)
    for bi in range(B):
        c, pk = _host_prep(events[offs[bi]:offs[bi + 1]])
        counts[bi] = c
        packs.append(pk)

    tiles_per_key = np.maximum(1, -(-counts.max(axis=0) // P)).astype(np.int64)
    for _ in range(12):
        T_tot = int(tiles_per_key.sum())
        col0 = np.zeros(NKEY + 1, np.int64)
        col0[1:] = np.cumsum(tiles_per_key)
        key_of_col = np.repeat(np.arange(NKEY), tiles_per_key)
        slots = [_assign_slots(pk, tiles_per_key) for pk in packs]
        base, width = _windows(slots, T_tot)
        over = width > 400     # x-window must fit a 512-col psum bank
        if not over.any():
            break
        bump = np.zeros(NKEY, bool)
        bump[key_of_col[over]] = True
        tiles_per_key[bump] *= 2

    WXM = int(width.max())
    key = (tuple(tiles_per_key.tolist()), tuple(base.tolist()),
           tuple(width.tolist()), os.environ.get("EVS_REPEAT", "1"))
    if key not in _prog_cache:
        _prog_cache[key] = _build_program(tiles_per_key, base, width, T_tot,
                                          WXM)
    nc = _prog_cache[key]

    in_maps = [_pack_core(sl, base, T_tot, key_of_col) for sl in slots]
    trace = bool(int(os.environ.get("EVS_TRACE", "0")))
    res = run_bass_kernel_spmd(nc, in_maps, core_ids=list(range(B)),
                               trace=trace)
    global last_results
    last_results = res
    out = np.stack([r["out"] for r in res.results], axis=0)
    return out.astype(np.float32)


last_results = None


if __name__ == "__main__":
    rng = np.random.default_rng(0)
    B0, NP0 = 8, 2000
    N0 = B0 * NP0
    x = rng.uniform(0, W - 1, N0).astype(np.float32)
    y = rng.uniform(0, H - 1, N0).astype(np.float32)
    t = np.sort(rng.uniform(0, 1, (B0, NP0)).astype(np.float32), axis=1).ravel()
    p = (2.0 * rng.integers(0, 2, N0) - 1).astype(np.float32)
    b = np.repeat(np.arange(B0), NP0).astype(np.float32)
    ev = np.stack([x, y, t, p, b], axis=1)
    ln = np.full(B0, NP0, np.int32)
    out = kernel(ev, ln)
    ref = np.zeros((B0, BINS, H, W), np.float64)
    for bi in range(B0):
        sl = slice(bi * NP0, (bi + 1) * NP0)
        xx, yy, tt2, pp = x[sl], y[sl], t[sl], p[sl]
        t0, tN = tt2[0], tt2[-1]
        ts = (BINS - 1) * np.clip((tt2 - t0) / (tN - t0), 0, 1)
        import itertools
        for xr_f, yr_f, br_f in itertools.product([np.floor, np.ceil], repeat=3):
            xr, yr, br = xr_f(xx), yr_f(yy), br_f(ts)
            valid = (((xr != xx) | (xr_f is np.floor))
                     & ((yr != yy) | (yr_f is np.floor))
                     & ((br != ts) | (br_f is np.floor))
                     & (xr < W) & (yr < H) & (br < BINS))
            kb = lambda a_: np.maximum(0, 1 - np.abs(a_))
            val = np.where(valid, pp * kb(xr - xx) * kb(yr - yy) * kb(br - ts), 0)
            np.add.at(ref[bi].ravel(),
                      np.where(valid, (xr + yr * W + br * H * W).astype(np.int64), 0),
                      val)
    err = np.abs(out - ref).max() / max(1e-9, np.abs(ref).max())
    print("smoke rel err:", err)


# revision 15
# speedup vs baseline: 1.0936x; 1.0936x over previous
"""EventVolumeSurface trilinear voxel-grid kernel for Trainium2 (Bass/Tile).

Strategy (data-parallel over batch, 1 batch -> 1 NeuronCore):
  Host: shard events by batch id, bucket by (time-segment s in [0,9),
  y-stripe q in [0,8) of 64 rows; events straddling a y-stripe boundary are
  duplicated - the trilinear hat masks the out-of-stripe tap), then SORT each
  bucket's events by x.  Cut into 128-event tiles; each tile gets a
  compile-time x-window [base_t, base_t+w_t) covering all 8 cores' taps
  (w_t ~ 16-40 cols).  Pack slot-major [128, T] arrays of y, x-base-relative,
  t, polarity.

  Device, per tile of 128 events (ops batched over groups of <=8 tiles):
    dy   = iota_y - y            (Pool,  [128, G*64])
    ady  = |dy|                  (ACT Abs)
    nhy  = min(ady,1) - 1        (DVE fused tensor_scalar, fp16) = -hat_y
    dx   = iota_w - xloc         (Pool/DVE)
    adx  = |dx|                  (ACT Abs)
    mxc  = min(adx,1)            (DVE, fp16)
    r0   = g0*mxc - g0           (DVE fused 2-scalar TS, fp16) = -g0*hat_x
    r1   = g1*mxc - g1           where g0 = p*(1-frac), g1 = p*frac
    psum[64, 1280] += nhy^T @ [r0 | r1]  (PE, N=w_t per bin half)
  so psum accumulates +g*hat_y*hat_x for the two adjacent bin planes.  PSUM
  is drained per (s,q) into an SBUF-resident [10,480,640] grid (pre-zeroed)
  which is DMA'd out plane-by-plane as planes finalize.
"""

import os
import sys

import numpy as np

sys.path.insert(0, "/opt/trn_rl_repo")

import concourse.bass as bass
import concourse.bacc as bacc
import concourse.mybir as mybir
import concourse.tile as tile
from concourse.bass_utils import run_bass_kernel_spmd

H, W, BINS = 480, 640, 10
NSEG = BINS - 1          # 9 time segments (events with t*=9 fold into seg 8)
P = 128
QS = 64                  # y-stripe height
NQ8 = 8                  # ceil(480/64) = 7.5 -> 8 stripes (last half-used)
NKEY = NSEG * NQ8        # 72 buckets
NQ4 = 4                  # V-grid column blocks of 128 rows
N_CORES = 8
GROUP = 8                # tiles per batched op
XCAP = 1280              # max f32 columns per batched x-op slab (SBUF budget)

F32 = mybir.dt.float32
F16 = mybir.dt.float16

# fraction (numerator/16) of per-tile r-ops issued on Pool engine (rest DVE)
R_POOL_NUM = int(os.environ.get("EVS_R_POOL_NUM", "5"))
# fraction (numerator/4) of plane drain-copies issued on DVE (rest ACT)
DR_DVE_NUM = int(os.environ.get("EVS_DR_DVE_NUM", "4"))

_prog_cache: dict = {}


def _host_prep(ev):
    """Bucket one batch's events by (s, q64); returns counts and raw data."""
    if ev.shape[0] == 0:
        ev = np.array([[0.0, 0.0, 0.25, 0.0, 0.0],
                       [0.0, 0.0, 0.75, 0.0, 0.0]], np.float32)
    x = ev[:, 0].astype(np.float32)
    y = ev[:, 1].astype(np.float32)
    t = ev[:, 2].astype(np.float32)
    p = ev[:, 3].astype(np.float32)
    t0 = t[0]
    tN = t[-1]
    denom = np.float32(tN - t0)
    if denom > 0:
        a = np.float32(np.float32(NSEG) / denom)
    else:
        a = np.float32(0.0)
    b = np.float32(-t0 * a)
    tp = (t * a + b).astype(np.float32)
    s = np.clip(np.floor(tp).astype(np.int32), 0, NSEG - 1)

    iy = np.floor(y).astype(np.int32)
    qf = iy >> 6
    qc = (iy + 1) >> 6
    n = len(x)
    idx0 = np.arange(n, dtype=np.int64)
    ys = qf != qc
    inst_idx = np.concatenate([idx0, idx0[ys]])
    inst_q = np.concatenate([qf, qc[ys]])
    key = s[inst_idx] * NQ8 + inst_q
    counts = np.bincount(key, minlength=NKEY)
    return counts, (x, y, t, p, a, b, inst_idx, key)


def _assign_slots(pack, tiles_per_key):
    """Sort instances by (key, x) and assign (partition, tile-col) slots."""
    x, y, t, p, a, b, inst_idx, key = pack
    col0 = np.zeros(NKEY + 1, np.int64)
    col0[1:] = np.cumsum(tiles_per_key)
    order = np.lexsort((x[inst_idx], key))
    skey = key[order]
    sidx = inst_idx[order]
    group_start = np.searchsorted(skey, np.arange(NKEY + 1))
    nk = np.diff(group_start)                      # this core's bucket counts
    rank = np.arange(len(skey)) - group_start[skey]
    # proportional-rank cut: tile j gets ranks [ceil(j*n/T), ceil((j+1)*n/T))
    # so each core's tiles cover aligned x-quantiles (narrower shared window)
    tk = tiles_per_key[skey]
    nks = np.maximum(nk[skey], 1)
    tile_in_key = (rank * tk) // nks
    # position within tile
    j0 = -(-(tile_in_key * nks) // tk)             # ceil(j*n/T)
    part = (rank - j0).astype(np.int64)
    col = (col0[skey] + tile_in_key).astype(np.int64)
    assert part.max(initial=0) < P
    return part, col, sidx, (x, y, t, p, a, b)


def _pack_core(slots, base, T_tot, key_of_col):
    part, col, sidx, (x, y, t, p, a, b) = slots
    xv = x[sidx]
    yv = y[sidx]
    tv = t[sidx]
    pv = p[sidx]

    qlo_col = ((key_of_col % NQ8) * QS).astype(np.float32)
    YLOC = np.zeros((P, T_tot), np.float32)
    YLOC[part, col] = yv - qlo_col[col]
    XLOC = np.zeros((P, T_tot), np.float32)
    XLOC[part, col] = xv - base[col].astype(np.float32)
    TP = np.zeros((P, 2 * T_tot + 2), np.float32)
    TP[part, col] = tv
    TP[part, T_tot + col] = pv
    TP[:, 2 * T_tot] = a
    TP[:, 2 * T_tot + 1] = b

    # K=3 fp16 lhsT slabs, rows [1, hi, lo], laid out [3, T*128] tile-major
    def split3(v):
        out = np.zeros((3, T_tot * P), np.float16)
        out[0] = 1.0
        hi = v.astype(np.float16)
        lo = (v - hi.astype(np.float32)).astype(np.float16)
        out[1].reshape(T_tot, P)[:] = hi.T
        out[2].reshape(T_tot, P)[:] = lo.T
        return out

    return {"ev_tp": TP, "yl3": split3(YLOC), "xl3": split3(XLOC)}


def _windows(slots, T_tot):
    """Shared per-tile x-window [base, base+width) covering all cores."""
    minx = np.full(T_tot, W, np.int64)
    maxe = np.zeros(T_tot, np.int64)
    for part, col, sidx, (x, y, t, p, a, b) in slots:
        fx = np.floor(x[sidx]).astype(np.int64)
        np.minimum.at(minx, col, fx)
        np.maximum.at(maxe, col, fx + 2)
    base = np.maximum(np.minimum(minx, W - 4), 0)
    end = np.minimum(np.maximum(maxe, base + 4), W)
    width = np.minimum((end - base + 1) // 2 * 2, W - base)
    return base, width


def _build_program(tiles_per_key, base, width, T_tot, WXM):
    nc = bacc.Bacc("TRN2", debug=False)
    tp_d = nc.dram_tensor("ev_tp", [P, 2 * T_tot + 2], F32,
                          kind="ExternalInput")
    yl3_d = nc.dram_tensor("yl3", [3, T_tot * P], F16, kind="ExternalInput")
    xl3_d = nc.dram_tensor("xl3", [3, T_tot * P], F16, kind="ExternalInput")
    out_d = nc.dram_tensor("out", [BINS, H, W], F32, kind="ExternalOutput")
    MAXT = int(tiles_per_key.max())

    col0 = np.zeros(NKEY + 1, np.int64)
    col0[1:] = np.cumsum(tiles_per_key)
    seg_c0 = [int(col0[s * NQ8]) for s in range(NSEG)]
    seg_c1 = [int(col0[(s + 1) * NQ8]) for s in range(NSEG)]

    Alu = mybir.AluOpType
    Act = mybir.ActivationFunctionType

    with tile.TileContext(nc) as tc:
        with (
            tc.tile_pool(name="persist", bufs=1) as persist,
            tc.tile_pool(name="psum",
                         bufs=int(os.environ.get("EVS_PL_BUFS", "2")),
                         space="PSUM") as psump,
        ):
            # K=3 matmul rhs constants: row0 = local iota (fp16 exact to
            # 2048), rows 1-2 = -1
            ioi = persist.tile([P, W], mybir.dt.int32, tag="ioi")
            nc.gpsimd.iota(ioi[:], pattern=[[1, W]], base=0,
                           channel_multiplier=0)
            rio = persist.tile([3, W], F16, tag="rio")
            nc.vector.memset(rio[:], -1.0)
            nc.vector.tensor_copy(rio[0:1, :], ioi[0:1, :])

            # prologue: frac = a*t + b - s ; g1 = p*frac ; g0 = p - g1
            g0p = persist.tile([P, T_tot], F32, tag="g0p")
            g1p = persist.tile([P, T_tot], F32, tag="g1p")
            with tc.tile_pool(name="prolog", bufs=1) as prolog:
                tpt = prolog.tile([P, 2 * T_tot + 2], F32, tag="tpt")
                tt = tpt[:, 0:T_tot]
                pt = tpt[:, T_tot:2 * T_tot]
                ab = tpt[:, 2 * T_tot:2 * T_tot + 2]
                nc.sync.dma_start(out=tpt[:], in_=tp_d[:])
                tc.strict_bb_all_engine_barrier()
                frac = prolog.tile([P, T_tot], F32, tag="frac")
                nc.vector.tensor_scalar(frac[:], tt, ab[:, 0:1], ab[:, 1:2],
                                        op0=Alu.mult, op1=Alu.add)
                for s in range(NSEG):
                    c0, c1 = seg_c0[s], seg_c1[s]
                    if c1 > c0 and s > 0:
                        nc.vector.tensor_scalar(frac[:, c0:c1], frac[:, c0:c1],
                                                float(s), None,
                                                op0=Alu.subtract)
                nc.vector.tensor_tensor(g1p[:], pt, frac[:], op=Alu.mult)
                nc.vector.tensor_tensor(g0p[:], pt, g1p[:], op=Alu.subtract)

            # output grid: every (plane, q8) region is first written by a
            # copy-drain, so no zero-init is needed
            z16 = persist.tile([P, 512], F16, tag="z16")
            nc.vector.memset(z16[:], 0.0)

            tc.strict_bb_all_engine_barrier()

            repeat = int(os.environ.get("EVS_REPEAT", "1"))
            r_rr = 0
            dr_rr = 0
            XBANKS = (0, 512, W)
            DG_BUFS = int(os.environ.get("EVS_DG_BUFS", "2"))
            with (
                tc.tile_pool(name="ylst", bufs=3) as ylstp,
                tc.tile_pool(name="xlst", bufs=3) as xlstp,
                tc.tile_pool(name="dyps", bufs=DG_BUFS, space="PSUM") as dypsp,
                tc.tile_pool(name="dxps", bufs=DG_BUFS, space="PSUM") as dxpsp,
                tc.tile_pool(name="adyp", bufs=4) as adyp,
                tc.tile_pool(name="nhyp", bufs=4) as nhyp,
                tc.tile_pool(name="adxp", bufs=4) as adxp,
                tc.tile_pool(name="mxcp", bufs=4) as mxcp,
                tc.tile_pool(name="rp", bufs=12) as rp,
                tc.tile_pool(name="stgp", bufs=3) as stgp,
            ):
             for _rep in range(repeat):
              for q8 in range(NQ8):
                pr0 = (q8 & 1) * QS
                rows = min(QS, H - q8 * QS)

                # plan all mm pieces for this q8 stripe: piece key =
                # (s, half, tile, p0, p1); per output plane+bank find the
                # last writer (gets stop=True) and which banks are untouched
                def pieces_of(s, half):
                    k = s * NQ8 + q8
                    out = []
                    for t_i in range(int(tiles_per_key[k])):
                        c = int(col0[k]) + t_i
                        w = int(width[c])
                        bs = int(base[c])
                        cur = bs
                        while cur < bs + w:
                            nxt_b = min(bs + w,
                                        next(b for b in XBANKS[1:] if b > cur))
                            out.append((t_i, cur, nxt_b))
                            cur = nxt_b
                    return out

                last_set = set()
                empty_banks = {}   # plane -> set of untouched bank indices
                for plane in range(BINS):
                    empty_banks[plane] = set()
                    for bk in range(2):
                        b0, b1 = XBANKS[bk], XBANKS[bk + 1]
                        writers = []
                        if plane >= 1:
                            writers += [(plane - 1, 1, t, p0, p1)
                                        for (t, p0, p1) in pieces_of(plane - 1, 1)
                                        if b0 <= p0 < b1]
                        if plane <= NSEG - 1:
                            writers += [(plane, 0, t, p0, p1)
                                        for (t, p0, p1) in pieces_of(plane, 0)
                                        if b0 <= p0 < b1]
                        if writers:
                            last_set.add(writers[-1])
                        else:
                            empty_banks[plane].add(bk)

                def new_plane_tile(plane):
                    ps = psump.tile([P, W], F32, tag="pp")
                    for bk in range(2):
                        b0, b1 = XBANKS[bk], XBANKS[bk + 1]
                        nc.tensor.matmul(ps[pr0:pr0 + QS, b0:b1],
                                         lhsT=z16[:, 0:QS],
                                         rhs=z16[:, 0:b1 - b0],
                                         start=True,
                                         stop=(bk in empty_banks[plane]))
                    return ps

                ptile = {0: new_plane_tile(0)}
                for s in range(NSEG):
                    ptile[s + 1] = new_plane_tile(s + 1)
                    k = s * NQ8 + q8
                    ntile = int(tiles_per_key[k])
                    cbase = int(col0[k])

                    # stream this bucket's lhsT slabs from DRAM
                    ylst = ylstp.tile([3, MAXT * P], F16, tag="ylst")
                    xlst = xlstp.tile([3, MAXT * P], F16, tag="xlst")
                    nc.sync.dma_start(
                        out=ylst[:, 0:ntile * P],
                        in_=yl3_d[:, cbase * P:(cbase + ntile) * P])
                    nc.sync.dma_start(
                        out=xlst[:, 0:ntile * P],
                        in_=xl3_d[:, cbase * P:(cbase + ntile) * P])

                    g0 = 0
                    while g0 < ntile:
                        gn = 1
                        wg = int(width[cbase + g0])
                        while (g0 + gn < ntile and gn < GROUP):
                            w2 = max(wg, int(width[cbase + g0 + gn]))
                            if (gn + 1) * w2 > 512:
                                break
                            wg = w2
                            gn += 1
                        gstart = g0
                        c0 = cbase + gstart
                        g0 += gn

                        # y side: dy (PE), ady16 (ACT from psum), nhy16 (DVE)
                        dyG = dypsp.tile([P, GROUP * QS], F32, tag="dyG")
                        for j in range(gn):
                            t_i = gstart + j
                            nc.tensor.matmul(
                                dyG[:, j * QS:(j + 1) * QS],
                                lhsT=ylst[:, t_i * P:(t_i + 1) * P],
                                rhs=rio[:, 0:QS], start=True, stop=True)
                        adyS = adyp.tile([P, GROUP * QS], F16, tag="adyS")
                        nc.scalar.activation(adyS[:, 0:gn * QS],
                                             dyG[:, 0:gn * QS], Act.Abs)
                        nhyS = nhyp.tile([P, GROUP * QS], F16, tag="nhyS")
                        nc.vector.tensor_scalar(nhyS[:, 0:gn * QS],
                                                adyS[:, 0:gn * QS], 1.0, 1.0,
                                                op0=Alu.min, op1=Alu.subtract)

                        # x side: dx (PE), adx16 (ACT from psum), mxc16 (DVE)
                        dxG = dxpsp.tile([P, 512], F32, tag="dxG")
                        for j in range(gn):
                            t_i = gstart + j
                            w = int(width[cbase + t_i])
                            nc.tensor.matmul(
                                dxG[:, j * wg:j * wg + w],
                                lhsT=xlst[:, t_i * P:(t_i + 1) * P],
                                rhs=rio[:, 0:w], start=True, stop=True)
                        adxS = adxp.tile([P, 512], F16, tag="adxS")
                        nc.scalar.activation(adxS[:, 0:gn * wg],
                                             dxG[:, 0:gn * wg], Act.Abs)
                        mxcS = mxcp.tile([P, 512], F16, tag="mxcS")
                        nc.vector.tensor_scalar(mxcS[:, 0:gn * wg],
                                                adxS[:, 0:gn * wg], 1.0, None,
                                                op0=Alu.min)

                        for j in range(gn):
                            t_i = gstart + j
                            c = cbase + t_i
                            w = int(width[c])
                            bs = int(base[c])
                            mx_j = mxcS[:, j * wg:j * wg + w]
                            for half, gcol in ((0, g0p), (1, g1p)):
                                ps = ptile[s + half]
                                rr_t = rp.tile([P, 640], F16, tag="rr")
                                rrw = rr_t[:, 0:w]
                                if r_rr < R_POOL_NUM:
                                    nc.gpsimd.tensor_scalar(
                                        rrw, mx_j, gcol[:, c:c + 1],
                                        gcol[:, c:c + 1],
                                        op0=Alu.mult, op1=Alu.subtract)
                                else:
                                    nc.vector.tensor_scalar(
                                        rrw, mx_j, gcol[:, c:c + 1],
                                        gcol[:, c:c + 1],
                                        op0=Alu.mult, op1=Alu.subtract)
                                r_rr = (r_rr + 1) & 15
                                cur = bs
                                while cur < bs + w:
                                    pe = min(bs + w,
                                             next(b for b in XBANKS[1:]
                                                  if b > cur))
                                    is_last = (s, half, t_i, cur, pe) \
                                        in last_set
                                    nc.tensor.matmul(
                                        ps[pr0:pr0 + QS, cur:pe],
                                        lhsT=nhyS[:, j * QS:(j + 1) * QS],
                                        rhs=rr_t[:, cur - bs:pe - bs],
                                        start=False, stop=is_last)
                                    cur = pe

                    # plane s complete: copy psum -> staging, DMA out
                    stg = stgp.tile([P, W], F32, tag="stg")
                    if dr_rr < DR_DVE_NUM:
                        nc.vector.tensor_copy(stg[pr0:pr0 + QS, :],
                                              ptile[s][pr0:pr0 + QS, :])
                    else:
                        nc.scalar.copy(stg[pr0:pr0 + QS, :],
                                       ptile[s][pr0:pr0 + QS, :])
                    dr_rr = (dr_rr + 1) & 3
                    if _rep == repeat - 1 and rows > 0:
                        nc.sync.dma_start(
                            out=out_d[s, q8 * QS:q8 * QS + rows, :],
                            in_=stg[pr0:pr0 + rows, :])
                    del ptile[s]

                stg = stgp.tile([P, W], F32, tag="stg")
                nc.scalar.copy(stg[pr0:pr0 + QS, :],
                               ptile[NSEG][pr0:pr0 + QS, :])
                if _rep == repeat - 1 and rows > 0:
                    nc.sync.dma_start(
                        out=out_d[NSEG, q8 * QS:q8 * QS + rows, :],
                        in_=stg[pr0:pr0 + rows, :])
                del ptile[NSEG]
    nc.finalize()
    return nc


def kernel(events, lengths):
    events = np.ascontiguousarray(events, dtype=np.float32)
    lengths = np.asarray(lengths)
    B = int(lengths.shape[0])
    offs = np.zeros(B + 1, np.int64)
    offs[1:] = np.cumsum(lengths)

    packs = []
    counts = np.zeros((B, NKEY), np.int64)
    for bi in range(B):
        c, pk = _host_prep(events[offs[bi]:offs[bi + 1]])
        counts[bi] = c
        packs.append(pk)

    tiles_per_key = np.maximum(1, -(-counts.max(axis=0) // P)).astype(np.int64)
    for _ in range(12):
        T_tot = int(tiles_per_key.sum())
        col0 = np.zeros(NKEY + 1, np.int64)
        col0[1:] = np.cumsum(tiles_per_key)
        key_of_col = np.repeat(np.arange(NKEY), tiles_per_key)
        slots = [_assign_slots(pk, tiles_per_key) for pk in packs]
        base, width = _windows(slots, T_tot)
        over = width > 400     # x-window must fit a 512-col psum bank
        if not over.any():
            break
        bump = np.zeros(NKEY, bool)
        bump[key_of_col[over]] = True
        tiles_per_key[bump] *= 2

    WXM = int(width.max())
    key = (tuple(tiles_per_key.tolist()), tuple(base.tolist()),
           tuple(width.tolist()), os.environ.get("EVS_REPEAT", "1"),
           R_POOL_NUM, DR_DVE_NUM, os.environ.get("EVS_PL_BUFS", "2"),
           os.environ.get("EVS_DG_BUFS", "2"))
    if key not in _prog_cache:
        _prog_cache[key] = _build_program(tiles_per_key, base, width, T_tot,
                                          WXM)
    nc = _prog_cache[key]

    in_maps = [_pack_core(sl, base, T_tot, key_of_col) for sl in slots]
    trace = bool(int(os.environ.get("EVS_TRACE", "0")))
    res = run_bass_kernel_spmd(nc, in_maps, core_ids=list(range(B)),
                               trace=trace)
    global last_results
    last_results = res
    out = np.stack([r["out"] for r in res.results], axis=0)
    return out.astype(np.float32)


last_results = None


if __name__ == "__main__":
    rng = np.random.default_rng(0)
    B0, NP0 = 8, 2000
    N0 = B0 * NP0
    x = rng.uniform(0, W - 1, N0).astype(np.float32)
    y = rng.uniform(0, H - 1, N0).astype(np.float32)
    t = np.sort(rng.uniform(0, 1, (B0, NP0)).astype(np.float32), axis=1).ravel()
    p = (2.0 * rng.integers(0, 2, N0) - 1).astype(np.float32)
    b = np.repeat(np.arange(B0), NP0).astype(np.float32)
    ev = np.stack([x, y, t, p, b], axis=1)
    ln = np.full(B0, NP0, np.int32)
    out = kernel(ev, ln)
    ref = np.zeros((B0, BINS, H, W), np.float64)
    for bi in range(B0):
        sl = slice(bi * NP0, (bi + 1) * NP0)
        xx, yy, tt2, pp = x[sl], y[sl], t[sl], p[sl]
        t0, tN = tt2[0], tt2[-1]
        ts = (BINS - 1) * np.clip((tt2 - t0) / (tN - t0), 0, 1)
        import itertools
        for xr_f, yr_f, br_f in itertools.product([np.floor, np.ceil], repeat=3):
            xr, yr, br = xr_f(xx), yr_f(yy), br_f(ts)
            valid = (((xr != xx) | (xr_f is np.floor))
                     & ((yr != yy) | (yr_f is np.floor))
                     & ((br != ts) | (br_f is np.floor))
                     & (xr < W) & (yr < H) & (br < BINS))
            kb = lambda a_: np.maximum(0, 1 - np.abs(a_))
            val = np.where(valid, pp * kb(xr - xx) * kb(yr - yy) * kb(br - ts), 0)
            np.add.at(ref[bi].ravel(),
                      np.where(valid, (xr + yr * W + br * H * W).astype(np.int64), 0),
                      val)
    err = np.abs(out - ref).max() / max(1e-9, np.abs(ref).max())
    print("smoke rel err:", err)
